# revision 17
# baseline (speedup 1.0000x reference)
"""C3D loss kernel for Trainium2 (8 NeuronCores, Bass/Tile) — v3.

Sharding: pure data parallel over B*2 = 8 shards (each image split into
top/bottom 176-row halves). Each core computes a partial numerator of the
loss; host combines and divides by the valid count.

Layout: partitions = 128 column blocks of 10 pixels (3+3 col halo -> 16
stored cols; blocks 122-127 pad past the image and are masked out).
Every spatial shift (5x5 window, normal central differences) is a
free-dim offset within a block's own storage. Host prepacks every input
into the exact contiguous [128, rows, 16] slab the DMA wants and
pre-scales depths by SQS so f16 intermediates stay in range.

Window phase is fused over the 5 dx offsets: one [128, 44, 5, 10] op per
(dy, row-half) computes all five dx shifts (the dx axis is a stride-1
overlapping window view; the center operand broadcasts via a stride-0
axis). Channel sums for d2 / normal-dot and the 25-offset accumulation
run as accumulating identity matmuls into PSUM (matmul cost is
moving-free-size only, so no channel stacking is needed).

Engine budget per offset: Pool does two of the f32 subs + all normal
gradient subs; Act does two squares + exp + abs; DVE does the rest
(f16 2x / TSP 4x paths). PE holds the channel/offset accumulation.

Out-of-image semantics match the reference's zero-pad + border mask:
normals come from zero-padded xyz; afterwards small strip DMAs poison
the out-of-image rows/cols of xp so exp underflows to exactly 0 there.
"""
import sys

sys.path.insert(0, "/opt/trn_rl_repo")

import numpy as np
from contextlib import ExitStack

import bass_rust
import concourse.bass as bass
import concourse.tile as tile
from concourse import bacc, mybir
from concourse.bass_utils import run_bass_kernel_spmd

F32 = mybir.dt.float32
F16 = mybir.dt.float16
AF = mybir.ActivationFunctionType
ALU = mybir.AluOpType

B, H, W = 4, 352, 1216
R = 2
EPS = 1e-8
N_CORES = 8

SH = H // 2          # shard rows per core = 176
NT = 2               # row tiles per core
TR = SH // NT        # output rows per tile = 88
HH = TR // 2         # PSUM chunk rows = 44
RB = TR + 6          # stored rows per tile = 94
CB = 10              # cols per block
NB = 128             # blocks (122 real + 6 pad)
BW = CB + 6          # stored cols per block = 16

SQS = 0.0625         # host pre-scale of depths (2^-4, exact)
EXS = float(200.0 / (SQS * SQS))   # exp scale compensation = 51200
PZV = 125.0          # poison value in scaled coords
QEPS = 1e-9          # rsqrt bias replacing the |n|+eps normalization

_prog_cache = {}


def _v(base_ap, dims, offset_elems):
    v = base_ap.copy()
    v.ap = bass_rust.VecI64Pair(dims)
    v.offset = v.offset + offset_elems
    return v


def _build_program():
    nc = bacc.Bacc("TRN2", target_bir_lowering=False, debug=False,
                   num_devices=N_CORES)

    for v in (QEPS,):
        ct = nc.alloc_sbuf_tensor(f"const-f32-{v}", [128, 1], F32)
        nc.gpsimd.memset(ct.ap(), v)
        nc.const_aps.aps[(F32, v)] = ct.ap()
    nc.all_engine_barrier()

    dp_d = nc.dram_tensor("dp", [NT, NB, RB, BW], F32, kind="ExternalInput").ap()
    dg_d = nc.dram_tensor("dg", [NT, NB, RB, BW], F32, kind="ExternalInput").ap()
    xy_d = nc.dram_tensor("xy1", [3, NT, NB, RB, BW], F32,
                          kind="ExternalInput").ap()
    mk_d = nc.dram_tensor("mk", [NT, NB, TR, CB], F16, kind="ExternalInput").ap()
    cs_d = nc.dram_tensor("cs", [RB, 2], F32, kind="ExternalInput").ap()
    rs_d = nc.dram_tensor("rs", [NT, 2, 3, NB, 2, BW], F32,
                          kind="ExternalInput").ap()
    id_d = nc.dram_tensor("idm", [NB, NB], F16, kind="ExternalInput").ap()
    out_d = nc.dram_tensor("out", [NB, NT], F32, kind="ExternalOutput").ap()

    with tile.TileContext(nc) as tc, ExitStack() as ctx:
        pool = ctx.enter_context(tc.tile_pool(name="p", bufs=1))
        psum = ctx.enter_context(tc.tile_pool(name="ps", bufs=1, space="PSUM"))
        idt = pool.tile([NB, NB], F16, name="idt")
        nc.sync.dma_start(out=idt[:], in_=id_d[:])

        for t in range(NT):
            # ---------------- input loads (contiguous) ----------------
            dpt = pool.tile([NB, RB, BW], F32, name="dpt")
            nc.sync.dma_start(out=dpt[:], in_=dp_d[t])
            dgt = pool.tile([NB, RB, BW], F32, name="dgt")
            nc.sync.dma_start(out=dgt[:], in_=dg_d[t])
            xy1t = [pool.tile([NB, RB, BW], F32, name=f"xy1t{c}") for c in range(3)]
            for c in range(3):
                nc.sync.dma_start(out=xy1t[c][:], in_=xy_d[c, t])
            mkt = pool.tile([NB, TR, CB], F16, name="mkt")
            nc.sync.dma_start(out=mkt[:], in_=mk_d[t])

            # ---------------- xyz (zero-padded; feeds normals) --------
            # xg first, then xp in place over xy1t (frees 18KB of SBUF)
            xg = [pool.tile([NB, RB, BW], F32, name=f"xg{c}") for c in range(3)]
            for c in range(3):
                nc.vector.tensor_mul(xg[c][:], xy1t[c][:], dgt[:])
            xp = xy1t
            for c in range(3):
                nc.vector.tensor_mul(xp[c][:], xy1t[c][:], dpt[:])

            # ---------------- normals (both keys, ln/exp batched) -----
            def grads(xc, key, rr, cc, nr, ncol, engs):
                def w(x, dr, dc):
                    return x[:, rr + dr:rr + dr + nr, cc + dc:cc + dc + ncol]
                gx = [pool.tile([NB, nr, ncol], F16, name=f"gx{key}{c}")
                      for c in range(3)]
                gy = [pool.tile([NB, nr, ncol], F16, name=f"gy{key}{c}")
                      for c in range(3)]
                for c in range(3):
                    engs[c].tensor_sub(gx[c][:], w(xc[c], 0, 1), w(xc[c], 0, -1))
                    engs[c].tensor_sub(gy[c][:], w(xc[c], 1, 0), w(xc[c], -1, 0))
                return gx, gy

            def cross_q(gx, gy, key, nr, ncol):
                cr = [pool.tile([NB, nr, ncol], F16, name=f"cr{key}{c}")
                      for c in range(3)]
                tA = pool.tile([NB, nr, ncol], F16, name=f"tA{key}")
                for c in range(3):
                    a, b = (c + 1) % 3, (c + 2) % 3
                    nc.vector.tensor_mul(cr[c][:], gx[a][:], gy[b][:])
                    nc.vector.tensor_mul(tA[:], gx[b][:], gy[a][:])
                    nc.vector.tensor_sub(cr[c][:], cr[c][:], tA[:])
                q = pool.tile([NB, nr, ncol], F16, name=f"q{key}")
                sqt = pool.tile([NB, nr, ncol], F16, name=f"sq{key}")
                nc.scalar.activation(q[:], cr[0][:], AF.Square)
                nc.scalar.activation(sqt[:], cr[1][:], AF.Square)
                nc.vector.tensor_add(q[:], q[:], sqt[:])
                nc.scalar.activation(sqt[:], cr[2][:], AF.Square)
                nc.vector.tensor_add(q[:], q[:], sqt[:])
                return cr, q

            gxp, gyp = grads(xp, "p", 1, 1, 92, 14,
                             [nc.gpsimd, nc.gpsimd, nc.gpsimd])
            gxg, gyg = grads(xg, "g", 3, 3, TR, CB,
                             [nc.vector, nc.vector, nc.gpsimd])

            # ------- poison xp borders (after grads read xp) -------
            for c in range(3):
                nc.sync.dma_start(out=xp[c][0:1, :, 1:3], in_=cs_d[:])
                nc.sync.dma_start(out=xp[c][121:122, :, 12:14], in_=cs_d[:])
                nc.sync.dma_start(out=xp[c][:, 1:3, :], in_=rs_d[t, 0, c])
                nc.sync.dma_start(out=xp[c][:, 91:93, :], in_=rs_d[t, 1, c])

            # ---------------- window phase ----------------
            # The first DEFER offsets' d2 path is emitted during the
            # normals tail so Pool/Act/PE keep working while the
            # cross-product chain runs; their normal-product path runs
            # right after the normals finish.
            accP = psum.tile([NB, 2, 512], F32, name="accP")
            DEFER = 6
            offs = [(dy, dx) for dy in range(-R, R + 1) for dx in range(-R, R + 1)]
            noff = len(offs)

            def shw(x, dy, dx):   # xp window view [NB, TR, CB]
                return x[:, 3 + dy:3 + dy + TR, 3 + dx:3 + dx + CB]

            def shn(x, dy, dx):   # ns window view [NB, TR, CB]
                return x[:, 2 + dy:2 + dy + TR, 2 + dx:2 + dx + CB]

            xgc = [xg[c][:, 3:3 + TR, 3:3 + CB] for c in range(3)]

            def d2_path(oi, kgt):
                dy, dx = offs[oi]
                d2P = psum.tile([NB, 2, 512], F32, name="d2P", tag="d2P",
                                bufs=2)
                sbs = [pool.tile([NB, TR, CB], F16, name=f"sbs{c}",
                                 tag=f"sbs{c}", bufs=2) for c in range(3)]
                sq = [pool.tile([NB, TR, CB], F16, name=f"sq{c}",
                                tag=f"sq{c}", bufs=2) for c in range(3)]
                nc.gpsimd.tensor_sub(sbs[0][:], shw(xp[0], dy, dx), xgc[0])
                nc.gpsimd.tensor_sub(sbs[1][:], shw(xp[1], dy, dx), xgc[1])
                nc.vector.tensor_sub(sbs[2][:], shw(xp[2], dy, dx), xgc[2])
                nc.scalar.activation(sq[0][:], sbs[0][:], AF.Square)
                nc.scalar.activation(sq[1][:], sbs[1][:], AF.Square)
                if oi % 2 == 0:
                    nc.vector.tensor_mul(sq[2][:], sbs[2][:], sbs[2][:])
                else:
                    nc.scalar.activation(sq[2][:], sbs[2][:], AF.Square)
                for c in range(3):
                    for ch in range(2):
                        rs = slice(ch * HH, (ch + 1) * HH)
                        nc.tensor.matmul(d2P[:, ch, 0:HH * CB]
                                         .rearrange("p (r c) -> p r c", c=CB),
                                         idt[:], sq[c][:, rs, :],
                                         start=(c == 0), stop=(c == 2))
                nc.scalar.activation(
                    kgt[:].rearrange("p (a r) c -> p a (r c)", a=2),
                    d2P[:, :, 0:HH * CB], AF.Exp, scale=-EXS)

            def nd_path(oi, kgt):
                dy, dx = offs[oi]
                ndP = psum.tile([NB, 2, 512], F32, name="ndP", tag="ndP")
                npr = [pool.tile([NB, TR, CB], F16, name=f"npr{c}",
                                 tag=f"npr{c}", bufs=2) for c in range(3)]
                stt = pool.tile([NB, TR, CB], F16, name="stt", tag="stt", bufs=2)
                trm = pool.tile([NB, TR, CB], F16, name="trm", tag="trm", bufs=2)
                for c in range(3):
                    nc.vector.tensor_mul(npr[c][:], shn(npn[c], dy, dx),
                                         ngn[c][:])
                for c in range(3):
                    for ch in range(2):
                        rs = slice(ch * HH, (ch + 1) * HH)
                        nc.tensor.matmul(ndP[:, ch, 0:HH * CB]
                                         .rearrange("p (r c) -> p r c", c=CB),
                                         idt[:], npr[c][:, rs, :],
                                         start=(c == 0), stop=(c == 2))
                nc.scalar.activation(
                    stt[:].rearrange("p (a r) c -> p a (r c)", a=2),
                    ndP[:, :, 0:HH * CB], AF.Abs, scale=1.9)
                nc.vector.tensor_scalar_add(stt[:], stt[:], 0.1)
                nc.vector.tensor_mul(trm[:], stt[:], kgt[:])
                for ch in range(2):
                    rs = slice(ch * HH, (ch + 1) * HH)
                    nc.tensor.matmul(accP[:, ch, 0:HH * CB]
                                     .rearrange("p (r c) -> p r c", c=CB),
                                     idt[:], trm[:, rs, :],
                                     start=(oi == 0), stop=(oi == noff - 1))

            kgtD = [pool.tile([NB, TR, CB], F16, name=f"kgtD{i}")
                    for i in range(DEFER)]
            for oi in range(DEFER):
                d2_path(oi, kgtD[oi])

            # normals tail (overlaps the deferred d2 work above)
            crp, qp = cross_q(gxp, gyp, "p", 92, 14)
            crg, qg = cross_q(gxg, gyg, "g", TR, CB)
            nc.scalar.activation(qp[:], qp[:], AF.Ln, bias=QEPS)
            nc.scalar.activation(qg[:], qg[:], AF.Ln, bias=QEPS)
            nc.scalar.activation(qp[:], qp[:], AF.Exp, scale=-0.5)
            nc.scalar.activation(qg[:], qg[:], AF.Exp, scale=-0.5)
            npn = [pool.tile([NB, 92, 14], F16, name=f"np{c}") for c in range(3)]
            ngn = [pool.tile([NB, TR, CB], F16, name=f"ng{c}") for c in range(3)]
            for c in range(3):
                nc.vector.tensor_mul(npn[c][:], crp[c][:], qp[:])
                nc.vector.tensor_mul(ngn[c][:], crg[c][:], qg[:])

            for oi in range(DEFER):
                nd_path(oi, kgtD[oi])
            for oi in range(DEFER, noff):
                kgt = pool.tile([NB, TR, CB], F16, name="kgt", tag="kgt", bufs=2)
                d2_path(oi, kgt)
                nd_path(oi, kgt)

            # ---------------- masked reduction ----------------
            mres = pool.tile([NB, TR, CB], F32, name="mres")
            nc.vector.tensor_mul(
                mres[:].rearrange("p (a r) c -> p a (r c)", a=2),
                accP[:, :, 0:HH * CB],
                mkt[:].rearrange("p (a r) c -> p a (r c)", a=2))
            red = pool.tile([NB, 1], F32, name="red")
            nc.vector.tensor_reduce(red[:], mres[:], mybir.AxisListType.XY,
                                    ALU.add)
            nc.sync.dma_start(out=out_d[0:NB, t:t + 1], in_=red[:])

    nc.compile()
    return nc


def _prepack(arr2d, t):
    """arr2d: padded canvas [SH+6, PW] (row 0 = image row r0-3, col 0 =
    image col -3) -> contiguous [NB, RB, BW] slab for tile t."""
    out = np.lib.stride_tricks.as_strided(
        arr2d[t * TR:],
        shape=(NB, RB, BW),
        strides=(CB * arr2d.strides[1], arr2d.strides[0], arr2d.strides[1]),
    )
    return np.ascontiguousarray(out)


def kernel(depth_pred, depth_gt, xy1_grid, K, mask):
    if "nc" not in _prog_cache:
        _prog_cache["nc"] = _build_program()
    nc = _prog_cache["nc"]

    dp = np.asarray(depth_pred, dtype=np.float32).reshape(B, H, W)
    dg = np.asarray(depth_gt, dtype=np.float32).reshape(B, H, W)
    xy1 = np.asarray(xy1_grid, dtype=np.float32)
    mk = np.asarray(mask).reshape(B, H, W)

    idm = np.eye(NB, dtype=np.float16)
    csc = np.full((RB, 2), PZV, dtype=np.float32)

    PW = NB * CB + BW + 8
    in_maps = []
    for core in range(N_CORES):
        b, half = core // 2, core % 2
        r0 = half * SH
        lo, hi = max(r0 - 3, 0), min(r0 + SH + 3, H)
        dpcv = np.zeros((SH + 6, PW), dtype=np.float32)
        dgcv = np.zeros((SH + 6, PW), dtype=np.float32)
        dpcv[lo - (r0 - 3):hi - (r0 - 3), 3:3 + W] = dp[b, lo:hi] * SQS
        dgcv[lo - (r0 - 3):hi - (r0 - 3), 3:3 + W] = dg[b, lo:hi] * SQS
        xycv = np.zeros((3, SH + 6, PW), dtype=np.float32)
        xycv[:, lo - (r0 - 3):hi - (r0 - 3), 3:3 + W] = xy1[b, :, lo:hi]
        mkcv = np.zeros((SH, PW), dtype=np.float16)
        mkcv[:, 3:3 + W] = mk[b, r0:r0 + SH]

        dp_t = np.stack([_prepack(dpcv, t) for t in range(NT)])
        dg_t = np.stack([_prepack(dgcv, t) for t in range(NT)])
        xy_t = np.stack([[_prepack(xycv[c], t) for t in range(NT)]
                         for c in range(3)])
        mk_t = np.zeros((NT, NB, TR, CB), dtype=np.float16)
        for t in range(NT):
            mk_t[t] = np.ascontiguousarray(
                mkcv[t * TR:(t + 1) * TR, 3:3 + NB * CB]
                .reshape(TR, NB, CB).transpose(1, 0, 2))

        # row-strip poison values: window-phase xp for slab rows 1:3 / 91:93.
        xpcv = xycv * dpcv[None]
        oob_row = np.zeros(SH + 6, dtype=bool)
        img_rows = np.arange(r0 - 3, r0 + SH + 3)
        oob_row[(img_rows < 0) | (img_rows >= H)] = True
        xpcv[:, oob_row, :] = PZV
        xpcv[:, :, 1:3] = PZV
        xpcv[:, :, 3 + W:3 + W + 2] = PZV
        rs_t = np.zeros((NT, 2, 3, NB, 2, BW), dtype=np.float32)
        for t in range(NT):
            for c in range(3):
                slab = _prepack(xpcv[c], t)
                rs_t[t, 0, c] = slab[:, 1:3, :]
                rs_t[t, 1, c] = slab[:, 91:93, :]

        in_maps.append({
            "dp": dp_t, "dg": dg_t, "xy1": xy_t, "mk": mk_t,
            "cs": csc, "rs": rs_t, "idm": idm,
        })

    res = run_bass_kernel_spmd(nc, in_maps, list(range(N_CORES)))
    total = 0.0
    for core in range(N_CORES):
        total += res.results[core]["out"].astype(np.float64).sum()
    nval = float(mk.sum(dtype=np.float64))
    return np.float32(-total / (nval + EPS))


# revision 19
# speedup vs baseline: 1.0090x; 1.0090x over previous
"""C3D loss kernel for Trainium2 (8 NeuronCores, Bass/Tile) — v3.

Sharding: pure data parallel over B*2 = 8 shards (each image split into
top/bottom 176-row halves). Each core computes a partial numerator of the
loss; host combines and divides by the valid count.

Layout: partitions = 128 column blocks of 10 pixels (3+3 col halo -> 16
stored cols; blocks 122-127 pad past the image and are masked out).
Every spatial shift (5x5 window, normal central differences) is a
free-dim offset within a block's own storage. Host prepacks every input
into the exact contiguous [128, rows, 16] slab the DMA wants and
pre-scales depths by SQS so f16 intermediates stay in range.

Window phase is fused over the 5 dx offsets: one [128, 44, 5, 10] op per
(dy, row-half) computes all five dx shifts (the dx axis is a stride-1
overlapping window view; the center operand broadcasts via a stride-0
axis). Channel sums for d2 / normal-dot and the 25-offset accumulation
run as accumulating identity matmuls into PSUM (matmul cost is
moving-free-size only, so no channel stacking is needed).

Engine budget per offset: Pool does two of the f32 subs + all normal
gradient subs; Act does two squares + exp + abs; DVE does the rest
(f16 2x / TSP 4x paths). PE holds the channel/offset accumulation.

Out-of-image semantics match the reference's zero-pad + border mask:
normals come from zero-padded xyz; afterwards small strip DMAs poison
the out-of-image rows/cols of xp so exp underflows to exactly 0 there.
"""
import sys

sys.path.insert(0, "/opt/trn_rl_repo")

import numpy as np
from contextlib import ExitStack

import bass_rust
import concourse.bass as bass
import concourse.tile as tile
from concourse import bacc, mybir
from concourse.bass_utils import run_bass_kernel_spmd

F32 = mybir.dt.float32
F16 = mybir.dt.float16
AF = mybir.ActivationFunctionType
ALU = mybir.AluOpType

B, H, W = 4, 352, 1216
R = 2
EPS = 1e-8
N_CORES = 8

SH = H // 2          # shard rows per core = 176
NT = 2               # row tiles per core
TR = SH // NT        # output rows per tile = 88
HH = TR // 2         # PSUM chunk rows = 44
RB = TR + 6          # stored rows per tile = 94
CB = 10              # cols per block
NB = 128             # blocks (122 real + 6 pad)
BW = CB + 6          # stored cols per block = 16

SQS = 0.0625         # host pre-scale of depths (2^-4, exact)
EXS = float(200.0 / (SQS * SQS))   # exp scale compensation = 51200
PZV = 125.0          # poison value in scaled coords
QEPS = 1e-9          # rsqrt bias replacing the |n|+eps normalization

_prog_cache = {}


def _v(base_ap, dims, offset_elems):
    v = base_ap.copy()
    v.ap = bass_rust.VecI64Pair(dims)
    v.offset = v.offset + offset_elems
    return v


def _build_program():
    nc = bacc.Bacc("TRN2", target_bir_lowering=False, debug=False,
                   num_devices=N_CORES)

    for v in (QEPS,):
        ct = nc.alloc_sbuf_tensor(f"const-f32-{v}", [128, 1], F32)
        nc.gpsimd.memset(ct.ap(), v)
        nc.const_aps.aps[(F32, v)] = ct.ap()
    nc.all_engine_barrier()

    dp_d = nc.dram_tensor("dp", [NT, NB, RB, BW], F32, kind="ExternalInput").ap()
    dg_d = nc.dram_tensor("dg", [NT, NB, RB, BW], F32, kind="ExternalInput").ap()
    xy_d = nc.dram_tensor("xy1", [3, NT, NB, RB, BW], F32,
                          kind="ExternalInput").ap()
    mk_d = nc.dram_tensor("mk", [NT, NB, TR, CB], F16, kind="ExternalInput").ap()
    cs_d = nc.dram_tensor("cs", [RB, 2], F32, kind="ExternalInput").ap()
    rs_d = nc.dram_tensor("rs", [NT, 2, 3, NB, 2, BW], F32,
                          kind="ExternalInput").ap()
    id_d = nc.dram_tensor("idm", [NB, NB], F16, kind="ExternalInput").ap()
    out_d = nc.dram_tensor("out", [NB, NT], F32, kind="ExternalOutput").ap()

    with tile.TileContext(nc) as tc, ExitStack() as ctx:
        pool = ctx.enter_context(tc.tile_pool(name="p", bufs=1))
        psum = ctx.enter_context(tc.tile_pool(name="ps", bufs=1, space="PSUM"))
        idt = pool.tile([NB, NB], F16, name="idt")
        nc.sync.dma_start(out=idt[:], in_=id_d[:])

        for t in range(NT):
            # ---------------- input loads (contiguous) ----------------
            dpt = pool.tile([NB, RB, BW], F32, name="dpt")
            nc.sync.dma_start(out=dpt[:], in_=dp_d[t])
            dgt = pool.tile([NB, RB, BW], F32, name="dgt")
            nc.sync.dma_start(out=dgt[:], in_=dg_d[t])
            xy1t = [pool.tile([NB, RB, BW], F32, name=f"xy1t{c}") for c in range(3)]
            for c in range(3):
                nc.sync.dma_start(out=xy1t[c][:], in_=xy_d[c, t])
            mkt = pool.tile([NB, TR, CB], F16, name="mkt")
            nc.sync.dma_start(out=mkt[:], in_=mk_d[t])

            # ---------------- xyz (zero-padded; feeds normals) --------
            xp = [pool.tile([NB, RB, BW], F32, name=f"xp{c}") for c in range(3)]
            xg = [pool.tile([NB, RB, BW], F32, name=f"xg{c}") for c in range(3)]
            for c in range(3):
                nc.vector.tensor_mul(xp[c][:], xy1t[c][:], dpt[:])

            # ---------------- normals (both keys, ln/exp batched) -----
            def grads(xc, key, rr, cc, nr, ncol, engs):
                def w(x, dr, dc):
                    return x[:, rr + dr:rr + dr + nr, cc + dc:cc + dc + ncol]
                gx = [pool.tile([NB, nr, ncol], F16, name=f"gx{key}{c}")
                      for c in range(3)]
                gy = [pool.tile([NB, nr, ncol], F16, name=f"gy{key}{c}")
                      for c in range(3)]
                for c in range(3):
                    engs[c].tensor_sub(gx[c][:], w(xc[c], 0, 1), w(xc[c], 0, -1))
                    engs[c].tensor_sub(gy[c][:], w(xc[c], 1, 0), w(xc[c], -1, 0))
                return gx, gy

            def cross_q(gx, gy, key, nr, ncol):
                cr = [pool.tile([NB, nr, ncol], F16, name=f"cr{key}{c}")
                      for c in range(3)]
                tA = pool.tile([NB, nr, ncol], F16, name=f"tA{key}")
                for c in range(3):
                    a, b = (c + 1) % 3, (c + 2) % 3
                    nc.vector.tensor_mul(cr[c][:], gx[a][:], gy[b][:])
                    nc.vector.tensor_mul(tA[:], gx[b][:], gy[a][:])
                    nc.vector.tensor_sub(cr[c][:], cr[c][:], tA[:])
                q = pool.tile([NB, nr, ncol], F16, name=f"q{key}")
                sqt = pool.tile([NB, nr, ncol], F16, name=f"sq{key}")
                nc.scalar.activation(q[:], cr[0][:], AF.Square)
                nc.scalar.activation(sqt[:], cr[1][:], AF.Square)
                nc.vector.tensor_add(q[:], q[:], sqt[:])
                nc.scalar.activation(sqt[:], cr[2][:], AF.Square)
                nc.vector.tensor_add(q[:], q[:], sqt[:])
                return cr, q

            # pred-side grads on Pool, channel by channel, each followed
            # immediately by that channel's border poison so the deferred
            # window subs unblock per channel.
            def wv(x, rr, cc, nr, ncol, dr, dc):
                return x[:, rr + dr:rr + dr + nr, cc + dc:cc + dc + ncol]

            gxp = [pool.tile([NB, 92, 14], F16, name=f"gxp{c}") for c in range(3)]
            gyp = [pool.tile([NB, 92, 14], F16, name=f"gyp{c}") for c in range(3)]
            for c in range(3):
                nc.gpsimd.tensor_sub(gxp[c][:], wv(xp[c], 1, 1, 92, 14, 0, 1),
                                     wv(xp[c], 1, 1, 92, 14, 0, -1))
                nc.gpsimd.tensor_sub(gyp[c][:], wv(xp[c], 1, 1, 92, 14, 1, 0),
                                     wv(xp[c], 1, 1, 92, 14, -1, 0))
                nc.sync.dma_start(out=xp[c][0:1, :, 1:3], in_=cs_d[:])
                nc.sync.dma_start(out=xp[c][121:122, :, 12:14], in_=cs_d[:])
                nc.sync.dma_start(out=xp[c][:, 1:3, :], in_=rs_d[t, 0, c])
                nc.sync.dma_start(out=xp[c][:, 91:93, :], in_=rs_d[t, 1, c])
            # gt xyz + grads fill the Vector engine meanwhile
            for c in range(3):
                nc.vector.tensor_mul(xg[c][:], xy1t[c][:], dgt[:])
            gxg, gyg = grads(xg, "g", 3, 3, TR, CB,
                             [nc.vector, nc.vector, nc.gpsimd])

            # ---------------- window phase ----------------
            # The first DEFER offsets' d2 path is emitted during the
            # normals tail so Pool/Act/PE keep working while the
            # cross-product chain runs; their normal-product path runs
            # right after the normals finish.
            accP = psum.tile([NB, 2, 512], F32, name="accP")
            DEFER = 6
            offs = [(dy, dx) for dy in range(-R, R + 1) for dx in range(-R, R + 1)]
            noff = len(offs)

            def shw(x, dy, dx):   # xp window view [NB, TR, CB]
                return x[:, 3 + dy:3 + dy + TR, 3 + dx:3 + dx + CB]

            def shn(x, dy, dx):   # ns window view [NB, TR, CB]
                return x[:, 2 + dy:2 + dy + TR, 2 + dx:2 + dx + CB]

            xgc = [xg[c][:, 3:3 + TR, 3:3 + CB] for c in range(3)]

            def d2_path(oi, kgt):
                dy, dx = offs[oi]
                d2P = psum.tile([NB, 2, 512], F32, name="d2P", tag="d2P",
                                bufs=2)
                sbs = [pool.tile([NB, TR, CB], F16, name=f"sbs{c}",
                                 tag=f"sbs{c}", bufs=2) for c in range(3)]
                sq = [pool.tile([NB, TR, CB], F16, name=f"sq{c}",
                                tag=f"sq{c}", bufs=2) for c in range(3)]
                nc.gpsimd.tensor_sub(sbs[0][:], shw(xp[0], dy, dx), xgc[0])
                nc.gpsimd.tensor_sub(sbs[1][:], shw(xp[1], dy, dx), xgc[1])
                nc.vector.tensor_sub(sbs[2][:], shw(xp[2], dy, dx), xgc[2])
                nc.scalar.activation(sq[0][:], sbs[0][:], AF.Square)
                nc.scalar.activation(sq[1][:], sbs[1][:], AF.Square)
                if oi % 2 == 0:
                    nc.vector.tensor_mul(sq[2][:], sbs[2][:], sbs[2][:])
                else:
                    nc.scalar.activation(sq[2][:], sbs[2][:], AF.Square)
                for c in range(3):
                    for ch in range(2):
                        rs = slice(ch * HH, (ch + 1) * HH)
                        nc.tensor.matmul(d2P[:, ch, 0:HH * CB]
                                         .rearrange("p (r c) -> p r c", c=CB),
                                         idt[:], sq[c][:, rs, :],
                                         start=(c == 0), stop=(c == 2))
                nc.scalar.activation(
                    kgt[:].rearrange("p (a r) c -> p a (r c)", a=2),
                    d2P[:, :, 0:HH * CB], AF.Exp, scale=-EXS)

            def nd_path(oi, kgt):
                dy, dx = offs[oi]
                ndP = psum.tile([NB, 2, 512], F32, name="ndP", tag="ndP")
                npr = [pool.tile([NB, TR, CB], F16, name=f"npr{c}",
                                 tag=f"npr{c}", bufs=2) for c in range(3)]
                stt = pool.tile([NB, TR, CB], F16, name="stt", tag="stt", bufs=2)
                trm = pool.tile([NB, TR, CB], F16, name="trm", tag="trm", bufs=2)
                for c in range(3):
                    nc.vector.tensor_mul(npr[c][:], shn(npn[c], dy, dx),
                                         ngn[c][:])
                for c in range(3):
                    for ch in range(2):
                        rs = slice(ch * HH, (ch + 1) * HH)
                        nc.tensor.matmul(ndP[:, ch, 0:HH * CB]
                                         .rearrange("p (r c) -> p r c", c=CB),
                                         idt[:], npr[c][:, rs, :],
                                         start=(c == 0), stop=(c == 2))
                nc.scalar.activation(
                    stt[:].rearrange("p (a r) c -> p a (r c)", a=2),
                    ndP[:, :, 0:HH * CB], AF.Abs, scale=1.9)
                nc.vector.tensor_scalar_add(stt[:], stt[:], 0.1)
                nc.vector.tensor_mul(trm[:], stt[:], kgt[:])
                for ch in range(2):
                    rs = slice(ch * HH, (ch + 1) * HH)
                    nc.tensor.matmul(accP[:, ch, 0:HH * CB]
                                     .rearrange("p (r c) -> p r c", c=CB),
                                     idt[:], trm[:, rs, :],
                                     start=(oi == 0), stop=(oi == noff - 1))

            kgtD = [pool.tile([NB, TR, CB], F16, name=f"kgtD{i}")
                    for i in range(DEFER)]
            for oi in range(DEFER):
                d2_path(oi, kgtD[oi])

            # normals tail (overlaps the deferred d2 work above)
            crp, qp = cross_q(gxp, gyp, "p", 92, 14)
            crg, qg = cross_q(gxg, gyg, "g", TR, CB)
            nc.scalar.activation(qp[:], qp[:], AF.Ln, bias=QEPS)
            nc.scalar.activation(qg[:], qg[:], AF.Ln, bias=QEPS)
            nc.scalar.activation(qp[:], qp[:], AF.Exp, scale=-0.5)
            nc.scalar.activation(qg[:], qg[:], AF.Exp, scale=-0.5)
            npn = [pool.tile([NB, 92, 14], F16, name=f"np{c}") for c in range(3)]
            ngn = [pool.tile([NB, TR, CB], F16, name=f"ng{c}") for c in range(3)]
            for c in range(3):
                nc.vector.tensor_mul(npn[c][:], crp[c][:], qp[:])
                nc.vector.tensor_mul(ngn[c][:], crg[c][:], qg[:])

            for oi in range(DEFER):
                nd_path(oi, kgtD[oi])
            for oi in range(DEFER, noff):
                kgt = pool.tile([NB, TR, CB], F16, name="kgt", tag="kgt", bufs=2)
                d2_path(oi, kgt)
                nd_path(oi, kgt)

            # ---------------- masked reduction ----------------
            mres = pool.tile([NB, TR, CB], F32, name="mres")
            nc.vector.tensor_mul(
                mres[:].rearrange("p (a r) c -> p a (r c)", a=2),
                accP[:, :, 0:HH * CB],
                mkt[:].rearrange("p (a r) c -> p a (r c)", a=2))
            red = pool.tile([NB, 1], F32, name="red")
            nc.vector.tensor_reduce(red[:], mres[:], mybir.AxisListType.XY,
                                    ALU.add)
            nc.sync.dma_start(out=out_d[0:NB, t:t + 1], in_=red[:])

    nc.compile()
    return nc


def _prepack(arr2d, t):
    """arr2d: padded canvas [SH+6, PW] (row 0 = image row r0-3, col 0 =
    image col -3) -> contiguous [NB, RB, BW] slab for tile t."""
    out = np.lib.stride_tricks.as_strided(
        arr2d[t * TR:],
        shape=(NB, RB, BW),
        strides=(CB * arr2d.strides[1], arr2d.strides[0], arr2d.strides[1]),
    )
    return np.ascontiguousarray(out)


def kernel(depth_pred, depth_gt, xy1_grid, K, mask):
    if "nc" not in _prog_cache:
        _prog_cache["nc"] = _build_program()
    nc = _prog_cache["nc"]

    dp = np.asarray(depth_pred, dtype=np.float32).reshape(B, H, W)
    dg = np.asarray(depth_gt, dtype=np.float32).reshape(B, H, W)
    xy1 = np.asarray(xy1_grid, dtype=np.float32)
    mk = np.asarray(mask).reshape(B, H, W)

    idm = np.eye(NB, dtype=np.float16)
    csc = np.full((RB, 2), PZV, dtype=np.float32)

    PW = NB * CB + BW + 8
    in_maps = []
    for core in range(N_CORES):
        b, half = core // 2, core % 2
        r0 = half * SH
        lo, hi = max(r0 - 3, 0), min(r0 + SH + 3, H)
        dpcv = np.zeros((SH + 6, PW), dtype=np.float32)
        dgcv = np.zeros((SH + 6, PW), dtype=np.float32)
        dpcv[lo - (r0 - 3):hi - (r0 - 3), 3:3 + W] = dp[b, lo:hi] * SQS
        dgcv[lo - (r0 - 3):hi - (r0 - 3), 3:3 + W] = dg[b, lo:hi] * SQS
        xycv = np.zeros((3, SH + 6, PW), dtype=np.float32)
        xycv[:, lo - (r0 - 3):hi - (r0 - 3), 3:3 + W] = xy1[b, :, lo:hi]
        mkcv = np.zeros((SH, PW), dtype=np.float16)
        mkcv[:, 3:3 + W] = mk[b, r0:r0 + SH]

        dp_t = np.stack([_prepack(dpcv, t) for t in range(NT)])
        dg_t = np.stack([_prepack(dgcv, t) for t in range(NT)])
        xy_t = np.stack([[_prepack(xycv[c], t) for t in range(NT)]
                         for c in range(3)])
        mk_t = np.zeros((NT, NB, TR, CB), dtype=np.float16)
        for t in range(NT):
            mk_t[t] = np.ascontiguousarray(
                mkcv[t * TR:(t + 1) * TR, 3:3 + NB * CB]
                .reshape(TR, NB, CB).transpose(1, 0, 2))

        # row-strip poison values: window-phase xp for slab rows 1:3 / 91:93.
        xpcv = xycv * dpcv[None]
        oob_row = np.zeros(SH + 6, dtype=bool)
        img_rows = np.arange(r0 - 3, r0 + SH + 3)
        oob_row[(img_rows < 0) | (img_rows >= H)] = True
        xpcv[:, oob_row, :] = PZV
        xpcv[:, :, 1:3] = PZV
        xpcv[:, :, 3 + W:3 + W + 2] = PZV
        rs_t = np.zeros((NT, 2, 3, NB, 2, BW), dtype=np.float32)
        for t in range(NT):
            for c in range(3):
                slab = _prepack(xpcv[c], t)
                rs_t[t, 0, c] = slab[:, 1:3, :]
                rs_t[t, 1, c] = slab[:, 91:93, :]

        in_maps.append({
            "dp": dp_t, "dg": dg_t, "xy1": xy_t, "mk": mk_t,
            "cs": csc, "rs": rs_t, "idm": idm,
        })

    res = run_bass_kernel_spmd(nc, in_maps, list(range(N_CORES)))
    total = 0.0
    for core in range(N_CORES):
        total += res.results[core]["out"].astype(np.float64).sum()
    nval = float(mk.sum(dtype=np.float64))
    return np.float32(-total / (nval + EPS))
